# revision 1
# baseline (speedup 1.0000x reference)
"""Trainium2 Bass kernel for nn_PerClassGating (moe_routing).

Computes, for inputs features[B,F], Ws[F,H], bs[H], W1[C,H,K], b1[C,K],
W2[C,K,E], b2[C,E] (B=256, F=2048, H=512, K=H/2=256, C=512, E=8):

    shared      = relu(features @ Ws + bs)                 # [B, H]
    h           = relu(einsum('bh,chk->bck', shared, W1) + b1)
    gate_logits = einsum('bck,cke->bce', h, W2) + b2       # [B, C, E]
    gate_weights = softmax(gate_logits, axis=-1)

Sharding: the class dim C is split across 8 NeuronCores (64 classes per
core); features and the shared transform are replicated. No collectives —
each core produces a disjoint [B, 64, E] slab of both outputs.

All matmul operands are float32r (fp32 rounded to the PE's replicated-
streaming format: ~4x plain-fp32 matmul throughput, ~19-bit mantissa,
fp32 accumulate in PSUM). The rounding happens for free inside SWDGE
(gpsimd) cast-DMAs for DRAM-sourced weights and inside the DVE
tensor_scalar epilogues for on-chip activations.

Host-side prep lays every tensor out so each DMA is a wide contiguous
transfer; see make_in_maps for the exact layouts.
"""

import numpy as np

B, F, H, C, E = 256, 2048, 512, 512, 8
K = H // 2  # 256
NCORES = 8
CPC = C // NCORES  # classes per core = 64
FC = F // 128      # 16 f-chunks
HC = H // 128      # 4 h-chunks
KC = K // 128      # 2 k-chunks
BATCH = 8          # classes per logits-PSUM batch
DMAGRP = 8         # classes per W1 DMA transfer (32 KiB/partition)
FGRP = 2           # f-chunks per shared-stage const DMA

_PROGRAM = None


def _build_program():
    from contextlib import ExitStack

    import concourse.bass as bass
    import concourse.mybir as mybir
    import concourse.tile as tile
    from concourse import bacc

    f32 = mybir.dt.float32
    f32r = mybir.dt.float32r
    Alu = mybir.AluOpType
    Act = mybir.ActivationFunctionType

    nc = bacc.Bacc(
        "TRN2", target_bir_lowering=False, debug=False, num_devices=NCORES
    )

    # fw: per f-chunk, featT[fc] (B cols) and ws[fc] (H cols) interleaved so
    # one DMA delivers matching moving+stationary data for a group of fcs.
    fw = nc.dram_tensor(
        "fw", [128, FC, B + H], f32, kind="ExternalInput"
    ).ap()
    w2 = nc.dram_tensor("w2", [128, CPC, KC, E], f32, kind="ExternalInput").ap()
    bs = nc.dram_tensor("bs", [128, HC], f32, kind="ExternalInput").ap()
    w1 = nc.dram_tensor(
        "w1", [128, CPC, HC, KC, 128], f32, kind="ExternalInput"
    ).ap()
    b1 = nc.dram_tensor("b1", [128, CPC, KC], f32, kind="ExternalInput").ap()
    b2 = nc.dram_tensor("b2", [128, CPC * E], f32, kind="ExternalInput").ap()
    out_lg = nc.dram_tensor(
        "out_logits", [B, CPC * E], f32, kind="ExternalOutput"
    ).ap()
    out_gw = nc.dram_tensor(
        "out_gw", [B, CPC * E], f32, kind="ExternalOutput"
    ).ap()

    NG = FC // FGRP  # 4 shared-stage DMA groups

    with tile.TileContext(nc) as tc, ExitStack() as ctx:
        const = ctx.enter_context(tc.tile_pool(name="const", bufs=1))
        spool = ctx.enter_context(tc.tile_pool(name="sharedT", bufs=1))
        w1pool = ctx.enter_context(tc.tile_pool(name="w1s", bufs=3))
        htpool = ctx.enter_context(tc.tile_pool(name="ht", bufs=3))
        outpool = ctx.enter_context(tc.tile_pool(name="outs", bufs=1))

        # ---- constant loads (SWDGE cast-DMAs round fp32 -> fp32r) ----------
        fwg_sb = []
        for g in range(NG):
            t = const.tile([128, FGRP, B + H], f32r, name=f"fwg{g}", tag=f"fwg{g}")
            nc.gpsimd.dma_start(out=t[:], in_=fw[:, g * FGRP : (g + 1) * FGRP, :])
            fwg_sb.append(t)
        w2_sb = const.tile([128, CPC, KC, E], f32r)
        bs_sb = const.tile([128, HC], f32)
        nc.sync.dma_start(out=bs_sb[:], in_=bs[:])
        b1_sb = const.tile([128, CPC, KC], f32)
        nc.sync.dma_start(out=b1_sb[:], in_=b1[:])
        b2_sb = const.tile([128, CPC * E], f32)
        nc.sync.dma_start(out=b2_sb[:], in_=b2[:])

        # ---- shared transform: sharedT[h, b] = relu(Ws.T @ featT + bs) ------
        # fc-major loop with four persistent PSUM banks (one per h-chunk) so
        # compute on DMA group g overlaps the load of group g+1.
        sh_sb = spool.tile([128, HC, B], f32r)
        with tc.tile_pool(name="ps_sh", bufs=1, space="PSUM") as ps_sh:
            ps_list = [
                ps_sh.tile([128, B], f32, name=f"pssh{hc}", tag=f"pssh{hc}")
                for hc in range(HC)
            ]
            # HAM warm-up: keep the PE busy while the first const DMAs land so
            # the clock gate opens (1.2 -> 2.4 GHz) before real work arrives
            warm_sb = const.tile([128, B], f32r, name="warm_sb")
            nc.vector.memset(warm_sb.bitcast(f32), 0.0)
            warm_ps = ps_sh.tile([128, B], f32, name="warm_ps", tag="dummy_ps", bufs=1)
            for i in range(90):
                nc.tensor.matmul(
                    warm_ps[:],
                    lhsT=warm_sb[:, :128],
                    rhs=warm_sb[:],
                    start=True,
                    stop=True,
                )
            for g in range(NG):
                for fl in range(FGRP):
                    for hc in range(HC):
                        nc.tensor.matmul(
                            ps_list[hc][:],
                            lhsT=fwg_sb[g][:, fl, B + 128 * hc : B + 128 * (hc + 1)],
                            rhs=fwg_sb[g][:, fl, :B],
                            start=(g == 0 and fl == 0),
                            stop=(g == NG - 1 and fl == FGRP - 1),
                        )
            for hc in range(HC):
                nc.vector.tensor_scalar(
                    out=sh_sb[:, hc, :],
                    in0=ps_list[hc][:],
                    scalar1=bs_sb[:, hc : hc + 1],
                    scalar2=0.0,
                    op0=Alu.add,
                    op1=Alu.max,
                )
            # wait-absorbers: one dummy matmul per sharedT chunk so the PE
            # observes every relu's DVE tick before the class loop
            dummy_ps = ps_sh.tile([128, B], f32, name="dummy_ps", bufs=1)
            for hc in range(HC):
                nc.tensor.matmul(
                    dummy_ps[:],
                    lhsT=fwg_sb[0][:, 0, B : B + 128],
                    rhs=sh_sb[:, hc, :],
                    start=True,
                    stop=True,
                )

        ps_ht = ctx.enter_context(
            tc.tile_pool(name="ps_ht", bufs=4, space="PSUM")
        )
        ps_lg = ctx.enter_context(
            tc.tile_pool(name="ps_lg", bufs=2, space="PSUM")
        )

        # ---- output accumulation tiles (SBUF-resident) ----------------------
        lg_sb = [outpool.tile([128, CPC * E], f32, name=f"lg{bc}", tag=f"lg{bc}") for bc in range(2)]
        gw_sb = [outpool.tile([128, CPC * E], f32, name=f"gw{bc}", tag=f"gw{bc}") for bc in range(2)]
        sums_sb = [outpool.tile([128, CPC], f32, name=f"sm{bc}", tag=f"sm{bc}") for bc in range(2)]
        rsum_sb = [outpool.tile([128, CPC], f32, name=f"rs{bc}", tag=f"rs{bc}") for bc in range(2)]

        # ---- per-class grouped GEMMs ---------------------------------------
        class_src = {}   # class -> (tile, idx)
        w2_emitted = False
        for batch in range(CPC // BATCH):
            ps_l = [ps_lg.tile([128, BATCH * E], f32, name=f"psl{bc}", tag=f"psl{bc}") for bc in range(2)]
            # stream this batch's W1 slab (32 KiB/partition per transfer).
            # The first two groups land as 2-class slices of one tile so the
            # earliest classes' weights arrive with the end of the shared
            # stage instead of after the whole 4 MiB group (Tile tracks deps
            # per-AP, so class c only waits on the slice DMA that carries it).
            for g in range(BATCH // DMAGRP):
                g0 = batch * BATCH + g * DMAGRP
                w1t = w1pool.tile([128, DMAGRP, HC, KC, 128], f32r, name="w1t")
                if batch < 2:
                    for q0 in range(0, DMAGRP, 2):
                        nc.gpsimd.dma_start(
                            out=w1t[:, q0 : q0 + 2],
                            in_=w1[:, g0 + q0 : g0 + q0 + 2],
                        )
                        if not w2_emitted:
                            nc.gpsimd.dma_start(out=w2_sb[:], in_=w2[:])
                            w2_emitted = True
                else:
                    nc.gpsimd.dma_start(out=w1t[:], in_=w1[:, g0 : g0 + DMAGRP])
                for j in range(DMAGRP):
                    class_src[g0 + j] = (w1t, j)
            def emit_l2(ci, ht):
                # layer 2: logits[b, e] = hT.T @ W2[c]  (accumulated over kc)
                c = batch * BATCH + ci
                for bc in range(2):
                    for kc in range(KC):
                        nc.tensor.matmul(
                            ps_l[bc][:, ci * E : (ci + 1) * E],
                            lhsT=ht[:, kc, bc * 128 : (bc + 1) * 128],
                            rhs=w2_sb[:, c, kc, :],
                            start=(kc == 0),
                            stop=(kc == KC - 1),
                        )

            pipelined = None  # (ci, ht) whose L2 is deferred one class
            for ci in range(BATCH):
                c = batch * BATCH + ci
                w1t, cg = class_src[c]
                # layer 1: hT[k, b] = relu(W1[c].T @ sharedT + b1[c])
                ht = htpool.tile([128, KC, B], f32r)
                for kc in range(KC):
                    ph = ps_ht.tile([128, B], f32)
                    for hc in range(HC):
                        nc.tensor.matmul(
                            ph[:],
                            lhsT=w1t[:, cg, hc, kc, :],
                            rhs=sh_sb[:, hc, :],
                            start=(hc == 0),
                            stop=(hc == HC - 1),
                        )
                    nc.vector.tensor_scalar(
                        out=ht[:, kc, :],
                        in0=ph[:],
                        scalar1=b1_sb[:, c, kc : kc + 1],
                        scalar2=0.0,
                        op0=Alu.add,
                        op1=Alu.max,
                    )
                # run the PREVIOUS class's layer 2 now: its relu finished long
                # ago, so the PE never waits on the DVE mid-stream
                if pipelined is not None:
                    emit_l2(*pipelined)
                pipelined = (ci, ht)
            emit_l2(*pipelined)
            # bias + exp + segmented row sums + normalize + store, all per
            # batch so the epilogue overlaps later batches' GEMMs
            lo, hi = batch * BATCH * E, (batch + 1) * BATCH * E
            cl, ch = batch * BATCH, (batch + 1) * BATCH
            for bc in range(2):
                nc.vector.tensor_add(
                    out=lg_sb[bc][:, lo:hi], in0=ps_l[bc][:], in1=b2_sb[:, lo:hi]
                )
                nc.scalar.activation(
                    out=gw_sb[bc][:, lo:hi],
                    in_=lg_sb[bc][:, lo:hi],
                    func=Act.Exp,
                )
                nc.vector.tensor_reduce(
                    out=sums_sb[bc][:, cl:ch],
                    in_=gw_sb[bc][:, lo:hi].rearrange("p (c e) -> p c e", e=E),
                    axis=mybir.AxisListType.X,
                    op=Alu.add,
                )
                nc.vector.reciprocal(
                    out=rsum_sb[bc][:, cl:ch], in_=sums_sb[bc][:, cl:ch]
                )
                rs = rsum_sb[bc][:, cl:ch]
                rs_bcast = bass.AP(
                    tensor=rs.tensor, offset=rs.offset, ap=[*rs.ap, [0, E]]
                )
                nc.vector.tensor_tensor(
                    out=gw_sb[bc][:, lo:hi].rearrange("p (c e) -> p c e", e=E),
                    in0=gw_sb[bc][:, lo:hi].rearrange("p (c e) -> p c e", e=E),
                    in1=rs_bcast,
                    op=Alu.mult,
                )
            if batch in (CPC // BATCH // 2 - 1, CPC // BATCH - 1):
                # flush outputs in two halves: big descriptors, and the mid-run
                # flush hides in a DMA lull while the tail flush stays small
                half = CPC * E // 2
                flo = 0 if batch < CPC // BATCH - 1 else half
                for bc in range(2):
                    nc.scalar.dma_start(
                        out=out_lg[bc * 128 : (bc + 1) * 128, flo : flo + half],
                        in_=lg_sb[bc][:, flo : flo + half],
                    )
                    nc.scalar.dma_start(
                        out=out_gw[bc * 128 : (bc + 1) * 128, flo : flo + half],
                        in_=gw_sb[bc][:, flo : flo + half],
                    )

    nc.compile()
    return nc


def get_program():
    global _PROGRAM
    if _PROGRAM is None:
        _PROGRAM = _build_program()
    return _PROGRAM


def make_in_maps(features, Ws, bs, W1, b1, W2, b2):
    """Host-side resharding of the full inputs into per-core device layouts."""
    f32 = np.float32
    features = np.ascontiguousarray(features, dtype=f32)
    Ws = np.ascontiguousarray(Ws, dtype=f32)
    bs = np.ascontiguousarray(bs, dtype=f32)
    W1 = np.ascontiguousarray(W1, dtype=f32)
    b1 = np.ascontiguousarray(b1, dtype=f32)
    W2 = np.ascontiguousarray(W2, dtype=f32)
    b2 = np.ascontiguousarray(b2, dtype=f32)

    featT_dev = features.T.reshape(FC, 128, B).transpose(1, 0, 2)  # [128,FC,B]
    ws_dev = Ws.reshape(FC, 128, H).transpose(1, 0, 2)             # [128,FC,H]
    fw_dev = np.ascontiguousarray(
        np.concatenate([featT_dev, ws_dev], axis=2)                # [128,FC,B+H]
    )
    bs_dev = np.ascontiguousarray(bs.reshape(HC, 128).T)

    in_maps = []
    for i in range(NCORES):
        c0 = i * CPC
        w1_dev = np.ascontiguousarray(
            W1[c0 : c0 + CPC]
            .reshape(CPC, HC, 128, KC, 128)
            .transpose(2, 0, 1, 3, 4)
        )
        b1_dev = np.ascontiguousarray(
            b1[c0 : c0 + CPC].reshape(CPC, KC, 128).transpose(2, 0, 1)
        )
        w2_dev = np.ascontiguousarray(
            W2[c0 : c0 + CPC].reshape(CPC, KC, 128, E).transpose(2, 0, 1, 3)
        )
        b2_dev = np.ascontiguousarray(
            np.broadcast_to(b2[c0 : c0 + CPC].reshape(1, CPC * E), (128, CPC * E))
        )
        in_maps.append(
            {
                "fw": fw_dev,
                "w2": w2_dev,
                "bs": bs_dev,
                "w1": w1_dev,
                "b1": b1_dev,
                "b2": b2_dev,
            }
        )
    return in_maps


def assemble(results):
    """Gather per-core [B, CPC*E] slabs into full [B, C, E] outputs."""
    gate_logits = np.empty((B, C, E), dtype=np.float32)
    gate_weights = np.empty((B, C, E), dtype=np.float32)
    for i, r in enumerate(results):
        c0 = i * CPC
        gate_logits[:, c0 : c0 + CPC, :] = r["out_logits"].reshape(B, CPC, E)
        gate_weights[:, c0 : c0 + CPC, :] = r["out_gw"].reshape(B, CPC, E)
    return gate_weights, gate_logits


def kernel(**inputs):
    from concourse.bass_utils import run_bass_kernel_spmd

    nc = get_program()
    in_maps = make_in_maps(**inputs)
    res = run_bass_kernel_spmd(nc, in_maps, core_ids=list(range(NCORES)))
    return assemble(res.results)



# revision 2
# speedup vs baseline: 1.3763x; 1.3763x over previous
"""Trainium2 Bass kernel for nn_PerClassGating (moe_routing).

Computes, for inputs features[B,F], Ws[F,H], bs[H], W1[C,H,K], b1[C,K],
W2[C,K,E], b2[C,E] (B=256, F=2048, H=512, K=H/2=256, C=512, E=8):

    shared      = relu(features @ Ws + bs)                 # [B, H]
    h           = relu(einsum('bh,chk->bck', shared, W1) + b1)
    gate_logits = einsum('bck,cke->bce', h, W2) + b2       # [B, C, E]
    gate_weights = softmax(gate_logits, axis=-1)

Sharding: the class dim C is split across 8 NeuronCores (64 classes per
core); features and the shared transform are replicated. No collectives —
each core produces a disjoint [B, 64, E] slab of both outputs.

All matmul operands are float16 (host-cast): 1 cycle/row on the PE at any
moving width, half the HBM traffic of fp32, and 128-cycle LDWEIGHTS (vs
256 for 32-bit stationaries). PSUM accumulation stays fp32, so the rel
err of the 3-GEMM chain stays ~1e-3, far under the 2e-2 gate.

Engine plan: PE streams the GEMMs; the per-class relu epilogues alternate
between ACT (kc=0, fused Relu(x+b1)) and DVE (kc=1, add+max) so neither
engine becomes the bottleneck; the tiny per-class L2 matmuls are emitted
one at a time between L1 matmuls so their weight loads hide under the
256-row L1 streams, and are pipelined one class behind (carried across
batch boundaries) so the PE never waits on a fresh relu. All input DMAs
ride one gpsimd ring in priority order (fw groups, then W2/biases, then
the W1 stream); outputs flush every two batches on the idle sync ring.
"""

import numpy as np

B, F, H, C, E = 256, 2048, 512, 512, 8
K = H // 2  # 256
NCORES = 8
CPC = C // NCORES  # classes per core = 64
FC = F // 128      # 16 f-chunks
HC = H // 128      # 4 h-chunks
KC = K // 128      # 2 k-chunks
BATCH = 8          # classes per logits-PSUM batch
DMAGRP = 8         # classes per W1 DMA transfer (16 KiB/partition)
FGRP = 2           # f-chunks per shared-stage const DMA
NWARM = 36         # PE clock-ramp warm-up matmuls

_PROGRAM = None


def _build_program():
    from contextlib import ExitStack

    import concourse.bass as bass
    import concourse.mybir as mybir
    import concourse.tile as tile
    from concourse import bacc

    f32 = mybir.dt.float32
    f16 = mybir.dt.float16
    Alu = mybir.AluOpType
    Act = mybir.ActivationFunctionType

    nc = bacc.Bacc(
        "TRN2", target_bir_lowering=False, debug=False, num_devices=NCORES
    )

    # fw: per f-chunk, featT[fc] (B cols) and ws[fc] (H cols) interleaved so
    # one DMA delivers matching moving+stationary data for a group of fcs.
    fw = nc.dram_tensor(
        "fw", [128, FC, B + H], f16, kind="ExternalInput"
    ).ap()
    w2 = nc.dram_tensor("w2", [128, CPC, KC, E], f16, kind="ExternalInput").ap()
    bs = nc.dram_tensor("bs", [128, HC], f32, kind="ExternalInput").ap()
    w1 = nc.dram_tensor(
        "w1", [128, CPC, HC, KC, 128], f16, kind="ExternalInput"
    ).ap()
    b1 = nc.dram_tensor("b1", [128, CPC, KC], f32, kind="ExternalInput").ap()
    b2 = nc.dram_tensor("b2", [128, CPC * E], f32, kind="ExternalInput").ap()
    out_lg = nc.dram_tensor(
        "out_logits", [B, CPC * E], f32, kind="ExternalOutput"
    ).ap()
    out_gw = nc.dram_tensor(
        "out_gw", [B, CPC * E], f32, kind="ExternalOutput"
    ).ap()

    NG = FC // FGRP  # 8 shared-stage DMA groups

    with tile.TileContext(nc) as tc, ExitStack() as ctx:
        const = ctx.enter_context(tc.tile_pool(name="const", bufs=1))
        spool = ctx.enter_context(tc.tile_pool(name="sharedT", bufs=1))
        w1pool = ctx.enter_context(tc.tile_pool(name="w1s", bufs=3))
        htpool = ctx.enter_context(tc.tile_pool(name="ht", bufs=3))
        outpool = ctx.enter_context(tc.tile_pool(name="outs", bufs=1))

        # ---- constant loads: one gpsimd ring, priority order ---------------
        fwg_sb = []
        for g in range(NG):
            t = const.tile([128, FGRP, B + H], f16, name=f"fwg{g}", tag=f"fwg{g}")
            nc.gpsimd.dma_start(out=t[:], in_=fw[:, g * FGRP : (g + 1) * FGRP, :])
            fwg_sb.append(t)
        w2_sb = const.tile([128, CPC, KC, E], f16)
        bs_sb = const.tile([128, HC], f32)
        nc.gpsimd.dma_start(out=bs_sb[:], in_=bs[:])
        b1_sb = const.tile([128, CPC, KC], f32)
        nc.gpsimd.dma_start(out=b1_sb[:], in_=b1[:])
        b2_sb = const.tile([128, CPC * E], f32)
        nc.gpsimd.dma_start(out=b2_sb[:], in_=b2[:])

        # ---- shared transform: sharedT[h, b] = relu(Ws.T @ featT + bs) ------
        # fc-major loop with four persistent PSUM banks (one per h-chunk) so
        # compute on DMA group g overlaps the load of group g+1.
        sh_sb = spool.tile([128, HC, B], f16)
        with tc.tile_pool(name="ps_sh", bufs=1, space="PSUM") as ps_sh:
            ps_list = [
                ps_sh.tile([128, B], f32, name=f"pssh{hc}", tag=f"pssh{hc}")
                for hc in range(HC)
            ]
            # HAM warm-up: keep the PE busy while the first const DMAs land so
            # the clock gate opens (1.2 -> 2.4 GHz) before real work arrives
            warm_sb = const.tile([128, B], f16, name="warm_sb")
            nc.vector.memset(warm_sb.bitcast(f32), 0.0)
            warm_ps = ps_sh.tile([128, B], f32, name="warm_ps", tag="dummy_ps", bufs=1)
            for i in range(NWARM):
                nc.tensor.matmul(
                    warm_ps[:],
                    lhsT=warm_sb[:, :128],
                    rhs=warm_sb[:],
                    start=True,
                    stop=True,
                )
            for g in range(NG):
                for fl in range(FGRP):
                    for hc in range(HC):
                        nc.tensor.matmul(
                            ps_list[hc][:],
                            lhsT=fwg_sb[g][:, fl, B + 128 * hc : B + 128 * (hc + 1)],
                            rhs=fwg_sb[g][:, fl, :B],
                            start=(g == 0 and fl == 0),
                            stop=(g == NG - 1 and fl == FGRP - 1),
                        )
            for hc in range(HC):
                if hc % 2 == 0:
                    nc.scalar.activation(
                        out=sh_sb[:, hc, :],
                        in_=ps_list[hc][:],
                        func=Act.Relu,
                        bias=bs_sb[:, hc : hc + 1],
                    )
                else:
                    nc.vector.tensor_scalar(
                        out=sh_sb[:, hc, :],
                        in0=ps_list[hc][:],
                        scalar1=bs_sb[:, hc : hc + 1],
                        scalar2=0.0,
                        op0=Alu.add,
                        op1=Alu.max,
                    )
            # wait-absorbers: one dummy matmul per sharedT chunk so the PE
            # observes every relu's ACT/DVE tick before the class loop
            dummy_ps = ps_sh.tile([128, B], f32, name="dummy_ps", bufs=1)
            for hc in range(HC):
                nc.tensor.matmul(
                    dummy_ps[:],
                    lhsT=fwg_sb[0][:, 0, B : B + 128],
                    rhs=sh_sb[:, hc, :],
                    start=True,
                    stop=True,
                )

        ps_ht = ctx.enter_context(
            tc.tile_pool(name="ps_ht", bufs=4, space="PSUM")
        )
        ps_lg = ctx.enter_context(
            tc.tile_pool(name="ps_lg", bufs=2, space="PSUM")
        )

        # ---- output accumulation tiles (SBUF-resident) ----------------------
        lg_sb = [outpool.tile([128, CPC * E], f32, name=f"lg{bc}", tag=f"lg{bc}") for bc in range(2)]
        gw_sb = [outpool.tile([128, CPC * E], f32, name=f"gw{bc}", tag=f"gw{bc}") for bc in range(2)]
        sums_sb = [outpool.tile([128, CPC], f32, name=f"sm{bc}", tag=f"sm{bc}") for bc in range(2)]
        rsum_sb = [outpool.tile([128, CPC], f32, name=f"rs{bc}", tag=f"rs{bc}") for bc in range(2)]

        # ---- per-class grouped GEMMs ---------------------------------------
        class_src = {}   # class -> (tile, idx)
        ps_l_all = {}    # batch -> [bc0_tile, bc1_tile]
        w2_emitted = False

        def emit_epilogue(batch):
            # bias + exp + segmented row sums + normalize, per batch so the
            # epilogue overlaps later batches' GEMMs
            lo, hi = batch * BATCH * E, (batch + 1) * BATCH * E
            cl, ch = batch * BATCH, (batch + 1) * BATCH
            ps_b = ps_l_all.pop(batch)
            for bc in range(2):
                nc.vector.tensor_add(
                    out=lg_sb[bc][:, lo:hi], in0=ps_b[bc][:], in1=b2_sb[:, lo:hi]
                )
                nc.scalar.activation(
                    out=gw_sb[bc][:, lo:hi],
                    in_=lg_sb[bc][:, lo:hi],
                    func=Act.Exp,
                )
                nc.vector.tensor_reduce(
                    out=sums_sb[bc][:, cl:ch],
                    in_=gw_sb[bc][:, lo:hi].rearrange("p (c e) -> p c e", e=E),
                    axis=mybir.AxisListType.X,
                    op=Alu.add,
                )
                nc.vector.reciprocal(
                    out=rsum_sb[bc][:, cl:ch], in_=sums_sb[bc][:, cl:ch]
                )
                rs = rsum_sb[bc][:, cl:ch]
                rs_bcast = bass.AP(
                    tensor=rs.tensor, offset=rs.offset, ap=[*rs.ap, [0, E]]
                )
                nc.vector.tensor_tensor(
                    out=gw_sb[bc][:, lo:hi].rearrange("p (c e) -> p c e", e=E),
                    in0=gw_sb[bc][:, lo:hi].rearrange("p (c e) -> p c e", e=E),
                    in1=rs_bcast,
                    op=Alu.mult,
                )
            if batch % 2 == 1:
                # flush two batches of both outputs on the idle sync ring:
                # 512 B descriptors, and the tail flush stays tiny
                flo = (batch - 1) * BATCH * E
                fsz = 2 * BATCH * E
                for bc in range(2):
                    nc.sync.dma_start(
                        out=out_lg[bc * 128 : (bc + 1) * 128, flo : flo + fsz],
                        in_=lg_sb[bc][:, flo : flo + fsz],
                    )
                    nc.sync.dma_start(
                        out=out_gw[bc * 128 : (bc + 1) * 128, flo : flo + fsz],
                        in_=gw_sb[bc][:, flo : flo + fsz],
                    )

        # pending L2 work: list of closures, one per (bc, kc) matmul of the
        # previous class, emitted singly between L1 matmuls so each LDWEIGHTS
        # hides under a 256-row L1 stream
        l2_queue = []

        def queue_l2(batch, ci, ht):
            c = batch * BATCH + ci
            ps_b = ps_l_all[batch]

            def mk(bc, kc):
                def emit():
                    nc.tensor.matmul(
                        ps_b[bc][:, ci * E : (ci + 1) * E],
                        lhsT=ht[:, kc, bc * 128 : (bc + 1) * 128],
                        rhs=w2_sb[:, c, kc, :],
                        start=(kc == 0),
                        stop=(kc == KC - 1),
                    )
                return emit

            for bc in range(2):
                for kc in range(KC):
                    l2_queue.append(mk(bc, kc))

        prev_batch_done = None  # batch whose epilogue is owed
        for batch in range(CPC // BATCH):
            ps_l_all[batch] = [
                ps_lg.tile([128, BATCH * E], f32, name=f"psl{bc}", tag=f"psl{bc}")
                for bc in range(2)
            ]
            # stream this batch's W1 slab. The first two groups land as
            # 2-class slices of one tile so the earliest classes' weights
            # arrive with the end of the shared stage instead of after the
            # whole 2 MiB group (Tile tracks deps per-AP, so class c only
            # waits on the slice DMA that carries it).
            for g in range(BATCH // DMAGRP):
                g0 = batch * BATCH + g * DMAGRP
                w1t = w1pool.tile([128, DMAGRP, HC, KC, 128], f16, name="w1t")
                if batch < 2:
                    for q0 in range(0, DMAGRP, 2):
                        nc.gpsimd.dma_start(
                            out=w1t[:, q0 : q0 + 2],
                            in_=w1[:, g0 + q0 : g0 + q0 + 2],
                        )
                        if not w2_emitted:
                            nc.gpsimd.dma_start(out=w2_sb[:], in_=w2[:])
                            w2_emitted = True
                else:
                    nc.gpsimd.dma_start(out=w1t[:], in_=w1[:, g0 : g0 + DMAGRP])
                for j in range(DMAGRP):
                    class_src[g0 + j] = (w1t, j)

            for ci in range(BATCH):
                c = batch * BATCH + ci
                w1t, cg = class_src[c]
                # layer 1: hT[k, b] = relu(W1[c].T @ sharedT + b1[c]), with
                # the previous class's L2 matmuls interleaved two per kc
                ht = htpool.tile([128, KC, B], f16)
                for kc in range(KC):
                    ph = ps_ht.tile([128, B], f32)
                    for hc in range(HC):
                        nc.tensor.matmul(
                            ph[:],
                            lhsT=w1t[:, cg, hc, kc, :],
                            rhs=sh_sb[:, hc, :],
                            start=(hc == 0),
                            stop=(hc == HC - 1),
                        )
                        if hc % 2 == 1 and l2_queue:
                            l2_queue.pop(0)()
                    if kc == 0:
                        nc.scalar.activation(
                            out=ht[:, kc, :],
                            in_=ph[:],
                            func=Act.Relu,
                            bias=b1_sb[:, c, kc : kc + 1],
                        )
                    else:
                        nc.vector.tensor_scalar(
                            out=ht[:, kc, :],
                            in0=ph[:],
                            scalar1=b1_sb[:, c, kc : kc + 1],
                            scalar2=0.0,
                            op0=Alu.add,
                            op1=Alu.max,
                        )
                queue_l2(batch, ci, ht)
                if ci == 0 and prev_batch_done is not None:
                    # the previous batch's last L2 matmuls just drained from
                    # l2_queue during this class's L1 — its PSUM is complete
                    emit_epilogue(prev_batch_done)
                    prev_batch_done = None
            prev_batch_done = batch
        # drain the final class's L2 (one unavoidable relu wait) + epilogue
        while l2_queue:
            l2_queue.pop(0)()
        emit_epilogue(prev_batch_done)

    nc.compile()
    return nc


def get_program():
    global _PROGRAM
    if _PROGRAM is None:
        _PROGRAM = _build_program()
    return _PROGRAM


def make_in_maps(features, Ws, bs, W1, b1, W2, b2):
    """Host-side resharding of the full inputs into per-core device layouts."""
    f32 = np.float32
    f16 = np.float16
    features = np.asarray(features, dtype=f32)
    Ws = np.asarray(Ws, dtype=f32)
    bs = np.ascontiguousarray(bs, dtype=f32)
    W1 = np.asarray(W1, dtype=f32)
    b1 = np.asarray(b1, dtype=f32)
    W2 = np.asarray(W2, dtype=f32)
    b2 = np.asarray(b2, dtype=f32)

    featT_dev = features.T.reshape(FC, 128, B).transpose(1, 0, 2)  # [128,FC,B]
    ws_dev = Ws.reshape(FC, 128, H).transpose(1, 0, 2)             # [128,FC,H]
    fw_dev = np.ascontiguousarray(
        np.concatenate([featT_dev, ws_dev], axis=2), dtype=f16     # [128,FC,B+H]
    )
    bs_dev = np.ascontiguousarray(bs.reshape(HC, 128).T)

    in_maps = []
    for i in range(NCORES):
        c0 = i * CPC
        w1_dev = np.ascontiguousarray(
            W1[c0 : c0 + CPC]
            .reshape(CPC, HC, 128, KC, 128)
            .transpose(2, 0, 1, 3, 4),
            dtype=f16,
        )
        b1_dev = np.ascontiguousarray(
            b1[c0 : c0 + CPC].reshape(CPC, KC, 128).transpose(2, 0, 1)
        )
        w2_dev = np.ascontiguousarray(
            W2[c0 : c0 + CPC].reshape(CPC, KC, 128, E).transpose(2, 0, 1, 3),
            dtype=f16,
        )
        b2_dev = np.ascontiguousarray(
            np.broadcast_to(b2[c0 : c0 + CPC].reshape(1, CPC * E), (128, CPC * E))
        )
        in_maps.append(
            {
                "fw": fw_dev,
                "w2": w2_dev,
                "bs": bs_dev,
                "w1": w1_dev,
                "b1": b1_dev,
                "b2": b2_dev,
            }
        )
    return in_maps


def assemble(results):
    """Gather per-core [B, CPC*E] slabs into full [B, C, E] outputs."""
    gate_logits = np.empty((B, C, E), dtype=np.float32)
    gate_weights = np.empty((B, C, E), dtype=np.float32)
    for i, r in enumerate(results):
        c0 = i * CPC
        gate_logits[:, c0 : c0 + CPC, :] = r["out_logits"].reshape(B, CPC, E)
        gate_weights[:, c0 : c0 + CPC, :] = r["out_gw"].reshape(B, CPC, E)
    return gate_weights, gate_logits


def kernel(**inputs):
    from concourse.bass_utils import run_bass_kernel_spmd

    nc = get_program()
    in_maps = make_in_maps(**inputs)
    res = run_bass_kernel_spmd(nc, in_maps, core_ids=list(range(NCORES)))
    return assemble(res.results)


# revision 7
# speedup vs baseline: 1.5290x; 1.1109x over previous
"""Trainium2 Bass kernel for nn_PerClassGating (moe_routing).

Computes, for inputs features[B,F], Ws[F,H], bs[H], W1[C,H,K], b1[C,K],
W2[C,K,E], b2[C,E] (B=256, F=2048, H=512, K=H/2=256, C=512, E=8):

    shared      = relu(features @ Ws + bs)                 # [B, H]
    h           = relu(einsum('bh,chk->bck', shared, W1) + b1)
    gate_logits = einsum('bck,cke->bce', h, W2) + b2       # [B, C, E]
    gate_weights = softmax(gate_logits, axis=-1)

Sharding: the class dim C is split across 8 NeuronCores (64 classes per
core); features and the shared transform are replicated. No collectives —
each core produces a disjoint [B, 64, E] slab of both outputs.

All matmul operands are float16 (host-cast): 1 cycle/row on the PE at any
moving width, half the HBM traffic of fp32, and 128-cycle LDWEIGHTS (vs
256 for 32-bit stationaries). PSUM accumulation stays fp32, so the rel
err of the 3-GEMM chain stays ~1e-3, far under the 2e-2 gate.

Engine plan: PE streams the GEMMs; the per-class relu epilogues alternate
between ACT (kc=0, fused Relu(x+b1)) and DVE (kc=1, add+max) so neither
engine becomes the bottleneck; the tiny per-class L2 matmuls are emitted
one at a time between L1 matmuls so their weight loads hide under the
256-row L1 streams, and are pipelined one class behind (carried across
batch boundaries) so the PE never waits on a fresh relu. All input DMAs
ride one gpsimd ring in priority order (fw groups, then W2/biases, then
the W1 stream); outputs flush every two batches on the idle sync ring.
"""

import numpy as np

B, F, H, C, E = 256, 2048, 512, 512, 8
K = H // 2  # 256
NCORES = 8
CPC = C // NCORES  # classes per core = 64
FC = F // 128      # 16 f-chunks
HC = H // 128      # 4 h-chunks
KC = K // 128      # 2 k-chunks
BATCH = 8          # classes per logits-PSUM batch
DMAGRP = 8         # classes per W1 DMA transfer (16 KiB/partition)
FGRP = 2           # f-chunks per shared-stage const DMA
NWARM = 36         # PE clock-ramp warm-up matmuls

_PROGRAM = None


def _build_program():
    from contextlib import ExitStack

    import concourse.bass as bass
    import concourse.mybir as mybir
    import concourse.tile as tile
    from concourse import bacc

    f32 = mybir.dt.float32
    f16 = mybir.dt.float16
    Alu = mybir.AluOpType
    Act = mybir.ActivationFunctionType

    nc = bacc.Bacc(
        "TRN2", target_bir_lowering=False, debug=False, num_devices=NCORES
    )

    # fw: per f-chunk, featT[fc] (B cols) and ws[fc] (H cols) interleaved so
    # one DMA delivers matching moving+stationary data for a group of fcs.
    fw = nc.dram_tensor(
        "fw", [128, FC, B + H], f16, kind="ExternalInput"
    ).ap()
    w2 = nc.dram_tensor("w2", [128, CPC, KC, E], f16, kind="ExternalInput").ap()
    bs = nc.dram_tensor("bs", [128, HC], f32, kind="ExternalInput").ap()
    w1 = nc.dram_tensor(
        "w1", [128, CPC, HC, KC, 128], f16, kind="ExternalInput"
    ).ap()
    b1 = nc.dram_tensor("b1", [128, CPC, KC], f32, kind="ExternalInput").ap()
    b2 = nc.dram_tensor("b2", [128, CPC * E], f32, kind="ExternalInput").ap()
    out_lg = nc.dram_tensor(
        "out_logits", [B, CPC * E], f32, kind="ExternalOutput"
    ).ap()
    out_gw = nc.dram_tensor(
        "out_gw", [B, CPC * E], f32, kind="ExternalOutput"
    ).ap()

    NG = FC // FGRP  # 8 shared-stage DMA groups

    with tile.TileContext(nc) as tc, ExitStack() as ctx:
        const = ctx.enter_context(tc.tile_pool(name="const", bufs=1))
        spool = ctx.enter_context(tc.tile_pool(name="sharedT", bufs=1))
        w1pool = ctx.enter_context(tc.tile_pool(name="w1s", bufs=3))
        htpool = ctx.enter_context(tc.tile_pool(name="ht", bufs=3))
        outpool = ctx.enter_context(tc.tile_pool(name="outs", bufs=1))

        # ---- constant loads: one gpsimd ring, priority order ---------------
        fwg_sb = []
        for g in range(NG):
            t = const.tile([128, FGRP, B + H], f16, name=f"fwg{g}", tag=f"fwg{g}")
            nc.gpsimd.dma_start(out=t[:], in_=fw[:, g * FGRP : (g + 1) * FGRP, :])
            fwg_sb.append(t)
        w2_sb = const.tile([128, CPC, KC, E], f16)
        bs_sb = const.tile([128, HC], f32)
        nc.gpsimd.dma_start(out=bs_sb[:], in_=bs[:])
        b1_sb = const.tile([128, CPC, KC], f32)
        nc.gpsimd.dma_start(out=b1_sb[:], in_=b1[:])
        b2_sb = const.tile([128, CPC * E], f32)
        nc.gpsimd.dma_start(out=b2_sb[:], in_=b2[:])

        # ---- shared transform: sharedT[h, b] = relu(Ws.T @ featT + bs) ------
        # fc-major loop with four persistent PSUM banks (one per h-chunk) so
        # compute on DMA group g overlaps the load of group g+1.
        sh_sb = spool.tile([128, HC, B], f16)
        with tc.tile_pool(name="ps_sh", bufs=1, space="PSUM") as ps_sh:
            ps_list = [
                ps_sh.tile([128, B], f32, name=f"pssh{hc}", tag=f"pssh{hc}")
                for hc in range(HC)
            ]
            # HAM warm-up: keep the PE busy while the first const DMAs land so
            # the clock gate opens (1.2 -> 2.4 GHz) before real work arrives
            warm_sb = const.tile([128, B], f16, name="warm_sb")
            nc.vector.memset(warm_sb.bitcast(f32), 0.0)
            # prime the ACT table (relu+exp live in one table) while the PE
            # warms up, so the first real relu doesn't eat the 1.3us load
            warm_act = const.tile([128, 1], f32, name="warm_act")
            nc.scalar.activation(
                out=warm_act[:], in_=warm_sb[:, :1], func=Act.Relu
            )
            nc.scalar.activation(
                out=warm_act[:], in_=warm_sb[:, :1], func=Act.Exp
            )
            warm_ps = ps_sh.tile([128, B], f32, name="warm_ps", tag="dummy_ps", bufs=1)
            for i in range(NWARM):
                nc.tensor.matmul(
                    warm_ps[:],
                    lhsT=warm_sb[:, :128],
                    rhs=warm_sb[:],
                    start=True,
                    stop=True,
                )
            for g in range(NG):
                for fl in range(FGRP):
                    for hc in range(HC):
                        nc.tensor.matmul(
                            ps_list[hc][:],
                            lhsT=fwg_sb[g][:, fl, B + 128 * hc : B + 128 * (hc + 1)],
                            rhs=fwg_sb[g][:, fl, :B],
                            start=(g == 0 and fl == 0),
                            stop=(g == NG - 1 and fl == FGRP - 1),
                        )
            for hc in range(HC):
                if hc % 2 == 0:
                    nc.scalar.activation(
                        out=sh_sb[:, hc, :],
                        in_=ps_list[hc][:],
                        func=Act.Relu,
                        bias=bs_sb[:, hc : hc + 1],
                    )
                else:
                    nc.vector.tensor_scalar(
                        out=sh_sb[:, hc, :],
                        in0=ps_list[hc][:],
                        scalar1=bs_sb[:, hc : hc + 1],
                        scalar2=0.0,
                        op0=Alu.add,
                        op1=Alu.max,
                    )
            # wait-absorbers: one dummy matmul per sharedT chunk so the PE
            # observes every relu's ACT/DVE tick before the class loop
            dummy_ps = ps_sh.tile([128, B], f32, name="dummy_ps", bufs=1)
            for hc in range(HC):
                nc.tensor.matmul(
                    dummy_ps[:],
                    lhsT=fwg_sb[0][:, 0, B : B + 128],
                    rhs=sh_sb[:, hc, :],
                    start=True,
                    stop=True,
                )

        ps_ht = ctx.enter_context(
            tc.tile_pool(name="ps_ht", bufs=4, space="PSUM")
        )
        ps_lg = ctx.enter_context(
            tc.tile_pool(name="ps_lg", bufs=2, space="PSUM")
        )

        # ---- output accumulation tiles (SBUF-resident) ----------------------
        lg_sb = [outpool.tile([128, CPC * E], f32, name=f"lg{bc}", tag=f"lg{bc}") for bc in range(2)]
        gw_sb = [outpool.tile([128, CPC * E], f32, name=f"gw{bc}", tag=f"gw{bc}") for bc in range(2)]
        sums_sb = [outpool.tile([128, CPC], f32, name=f"sm{bc}", tag=f"sm{bc}") for bc in range(2)]
        rsum_sb = [outpool.tile([128, CPC], f32, name=f"rs{bc}", tag=f"rs{bc}") for bc in range(2)]

        # ---- per-class grouped GEMMs ---------------------------------------
        class_src = {}   # class -> (tile, idx)
        ps_l_all = {}    # batch -> [bc0_tile, bc1_tile]
        w2_emitted = False

        def emit_epilogue(batch):
            # bias + exp + segmented row sums + normalize, per batch so the
            # epilogue overlaps later batches' GEMMs
            lo, hi = batch * BATCH * E, (batch + 1) * BATCH * E
            cl, ch = batch * BATCH, (batch + 1) * BATCH
            ps_b = ps_l_all.pop(batch)
            for bc in range(2):
                nc.vector.tensor_add(
                    out=lg_sb[bc][:, lo:hi], in0=ps_b[bc][:], in1=b2_sb[:, lo:hi]
                )
                nc.scalar.activation(
                    out=gw_sb[bc][:, lo:hi],
                    in_=lg_sb[bc][:, lo:hi],
                    func=Act.Exp,
                )
                nc.vector.tensor_reduce(
                    out=sums_sb[bc][:, cl:ch],
                    in_=gw_sb[bc][:, lo:hi].rearrange("p (c e) -> p c e", e=E),
                    axis=mybir.AxisListType.X,
                    op=Alu.add,
                )
                nc.vector.reciprocal(
                    out=rsum_sb[bc][:, cl:ch], in_=sums_sb[bc][:, cl:ch]
                )
                rs = rsum_sb[bc][:, cl:ch]
                rs_bcast = bass.AP(
                    tensor=rs.tensor, offset=rs.offset, ap=[*rs.ap, [0, E]]
                )
                nc.vector.tensor_tensor(
                    out=gw_sb[bc][:, lo:hi].rearrange("p (c e) -> p c e", e=E),
                    in0=gw_sb[bc][:, lo:hi].rearrange("p (c e) -> p c e", e=E),
                    in1=rs_bcast,
                    op=Alu.mult,
                )
            if batch % 2 == 1:
                # flush two batches of both outputs on the idle sync ring:
                # 512 B descriptors, and the tail flush stays tiny
                flo = (batch - 1) * BATCH * E
                fsz = 2 * BATCH * E
                for bc in range(2):
                    nc.sync.dma_start(
                        out=out_lg[bc * 128 : (bc + 1) * 128, flo : flo + fsz],
                        in_=lg_sb[bc][:, flo : flo + fsz],
                    )
                    nc.sync.dma_start(
                        out=out_gw[bc * 128 : (bc + 1) * 128, flo : flo + fsz],
                        in_=gw_sb[bc][:, flo : flo + fsz],
                    )

        # pending L2 work: list of closures, one per (bc, kc) matmul of the
        # previous class, emitted singly between L1 matmuls so each LDWEIGHTS
        # hides under a 256-row L1 stream
        l2_queue = []

        def queue_l2(batch, ci, ht):
            c = batch * BATCH + ci
            ps_b = ps_l_all[batch]

            def mk(bc, kc):
                def emit():
                    nc.tensor.matmul(
                        ps_b[bc][:, ci * E : (ci + 1) * E],
                        lhsT=ht[:, kc, bc * 128 : (bc + 1) * 128],
                        rhs=w2_sb[:, c, kc, :],
                        start=(kc == 0),
                        stop=(kc == KC - 1),
                    )
                return emit

            for bc in range(2):
                for kc in range(KC):
                    l2_queue.append(mk(bc, kc))

        prev_batch_done = None  # batch whose epilogue is owed
        for batch in range(CPC // BATCH):
            ps_l_all[batch] = [
                ps_lg.tile([128, BATCH * E], f32, name=f"psl{bc}", tag=f"psl{bc}")
                for bc in range(2)
            ]
            # stream this batch's W1 slab. The first two groups land as
            # 2-class slices of one tile so the earliest classes' weights
            # arrive with the end of the shared stage instead of after the
            # whole 2 MiB group (Tile tracks deps per-AP, so class c only
            # waits on the slice DMA that carries it).
            for g in range(BATCH // DMAGRP):
                g0 = batch * BATCH + g * DMAGRP
                w1t = w1pool.tile([128, DMAGRP, HC, KC, 128], f16, name="w1t")
                if batch < 2:
                    for q0 in range(0, DMAGRP, 2):
                        nc.gpsimd.dma_start(
                            out=w1t[:, q0 : q0 + 2],
                            in_=w1[:, g0 + q0 : g0 + q0 + 2],
                        )
                        if not w2_emitted:
                            nc.gpsimd.dma_start(out=w2_sb[:], in_=w2[:])
                            w2_emitted = True
                else:
                    nc.gpsimd.dma_start(out=w1t[:], in_=w1[:, g0 : g0 + DMAGRP])
                for j in range(DMAGRP):
                    class_src[g0 + j] = (w1t, j)

            for ci in range(BATCH):
                c = batch * BATCH + ci
                w1t, cg = class_src[c]
                # layer 1: hT[k, b] = relu(W1[c].T @ sharedT + b1[c]), with
                # the previous class's L2 matmuls interleaved two per kc
                ht = htpool.tile([128, KC, B], f16)
                for kc in range(KC):
                    ph = ps_ht.tile([128, B], f32)
                    for hc in range(HC):
                        nc.tensor.matmul(
                            ph[:],
                            lhsT=w1t[:, cg, hc, kc, :],
                            rhs=sh_sb[:, hc, :],
                            start=(hc == 0),
                            stop=(hc == HC - 1),
                        )
                        if hc % 2 == 1 and len(l2_queue) > 4:
                            # keep 4 entries queued: L2 runs two classes
                            # behind its relu, so it never waits on ACT/DVE
                            l2_queue.pop(0)()
                    if kc == 0:
                        nc.scalar.activation(
                            out=ht[:, kc, :],
                            in_=ph[:],
                            func=Act.Relu,
                            bias=b1_sb[:, c, kc : kc + 1],
                        )
                    else:
                        nc.vector.tensor_scalar(
                            out=ht[:, kc, :],
                            in0=ph[:],
                            scalar1=b1_sb[:, c, kc : kc + 1],
                            scalar2=0.0,
                            op0=Alu.add,
                            op1=Alu.max,
                        )
                queue_l2(batch, ci, ht)
                if ci == 1 and prev_batch_done is not None:
                    # the previous batch's last L2 matmuls just drained from
                    # l2_queue during this class's L1 — its PSUM is complete
                    emit_epilogue(prev_batch_done)
                    prev_batch_done = None
            prev_batch_done = batch
        # drain the final class's L2 (one unavoidable relu wait) + epilogue
        while l2_queue:
            l2_queue.pop(0)()
        emit_epilogue(prev_batch_done)

    nc.compile()
    return nc


def get_program():
    global _PROGRAM
    if _PROGRAM is None:
        _PROGRAM = _build_program()
    return _PROGRAM


def make_in_maps(features, Ws, bs, W1, b1, W2, b2):
    """Host-side resharding of the full inputs into per-core device layouts."""
    f32 = np.float32
    f16 = np.float16
    features = np.asarray(features, dtype=f32)
    Ws = np.asarray(Ws, dtype=f32)
    bs = np.ascontiguousarray(bs, dtype=f32)
    W1 = np.asarray(W1, dtype=f32)
    b1 = np.asarray(b1, dtype=f32)
    W2 = np.asarray(W2, dtype=f32)
    b2 = np.asarray(b2, dtype=f32)

    featT_dev = features.T.reshape(FC, 128, B).transpose(1, 0, 2)  # [128,FC,B]
    ws_dev = Ws.reshape(FC, 128, H).transpose(1, 0, 2)             # [128,FC,H]
    fw_dev = np.ascontiguousarray(
        np.concatenate([featT_dev, ws_dev], axis=2), dtype=f16     # [128,FC,B+H]
    )
    bs_dev = np.ascontiguousarray(bs.reshape(HC, 128).T)

    in_maps = []
    for i in range(NCORES):
        c0 = i * CPC
        w1_dev = np.ascontiguousarray(
            W1[c0 : c0 + CPC]
            .reshape(CPC, HC, 128, KC, 128)
            .transpose(2, 0, 1, 3, 4),
            dtype=f16,
        )
        b1_dev = np.ascontiguousarray(
            b1[c0 : c0 + CPC].reshape(CPC, KC, 128).transpose(2, 0, 1)
        )
        w2_dev = np.ascontiguousarray(
            W2[c0 : c0 + CPC].reshape(CPC, KC, 128, E).transpose(2, 0, 1, 3),
            dtype=f16,
        )
        b2_dev = np.ascontiguousarray(
            np.broadcast_to(b2[c0 : c0 + CPC].reshape(1, CPC * E), (128, CPC * E))
        )
        in_maps.append(
            {
                "fw": fw_dev,
                "w2": w2_dev,
                "bs": bs_dev,
                "w1": w1_dev,
                "b1": b1_dev,
                "b2": b2_dev,
            }
        )
    return in_maps


def assemble(results):
    """Gather per-core [B, CPC*E] slabs into full [B, C, E] outputs."""
    gate_logits = np.empty((B, C, E), dtype=np.float32)
    gate_weights = np.empty((B, C, E), dtype=np.float32)
    for i, r in enumerate(results):
        c0 = i * CPC
        gate_logits[:, c0 : c0 + CPC, :] = r["out_logits"].reshape(B, CPC, E)
        gate_weights[:, c0 : c0 + CPC, :] = r["out_gw"].reshape(B, CPC, E)
    return gate_weights, gate_logits


def kernel(**inputs):
    from concourse.bass_utils import run_bass_kernel_spmd

    nc = get_program()
    in_maps = make_in_maps(**inputs)
    res = run_bass_kernel_spmd(nc, in_maps, core_ids=list(range(NCORES)))
    return assemble(res.results)
